# revision 5
# baseline (speedup 1.0000x reference)
"""MoE FFN (8 experts, top-2) on 8 TRN2 NeuronCores, expert-parallel.

Strategy:
  - Host: router (fp64 logits -> softmax -> top-2 -> renormalized combine
    weights), gather each expert's assigned tokens, pad to a common
    capacity C (SPMD: one program, per-core inputs).
  - Core e: full SwiGLU FFN for expert e over its C tokens in bf16
    (fp32 PSUM accumulation), combine-weight scaling on device;
    outputs [C, 1024] bf16.
  - Host: scatter-add per-expert outputs back into [B, S, D].

All tensors bf16 (halves DMA vs fp32r at the same PE rate; rel err
~4.4e-3 vs the fp32 reference, tolerance 2e-2). down_w stays fully
SBUF-resident across the rep loop (64KB/partition); gate/up weights
stream per chunk. Output DMAs ride the Activation HWDGE queue to keep
the SP queue free for weight prefetch.

Layouts (host-prepared, DMA-friendly):
  xT   [8, 128, C]       x[idx].T split along d into 8 k-tiles
  gw/uw[32, 128, 8, 128] gate/up ^T tiled: [h_tile][d_sub][k][h]
  dw   [32, 128, 1024]   down^T tiled:     [h_tile][h_sub][dout]
  cwT  [128, ceil(C/128)] combine weights, partition-major
"""
import sys, os
for p in ("/opt/trn_rl_repo", os.path.join(os.path.dirname(os.path.abspath(__file__)))):
    if p not in sys.path:
        sys.path.insert(0, p)
import numpy as np
import ml_dtypes

BF16 = ml_dtypes.bfloat16
D_MODEL = 1024
D_INNER = 4096
N_EXPERTS = 8
TOP_K = 2
H_TILES = D_INNER // 128  # 32
K_TILES = D_MODEL // 128  # 8
MAXCH = 1056              # token chunk cap (hbuf SBUF limit)


def _capacity(max_n: int) -> int:
    return max(256, ((max_n + 31) // 32) * 32)


def _chunks_of(C: int):
    # every chunk except the last must be a multiple of 128 so chunk starts
    # stay 128-aligned (cw column mapping + phase2 sub tiling rely on it)
    n = -(-C // MAXCH)
    while True:
        base = (C // n) // 128 * 128
        last = C - base * (n - 1)
        if 0 < last <= MAXCH:
            break
        n += 1
    out = [base] * (n - 1) + [last]
    assert sum(out) == C and all(0 < t <= MAXCH for t in out)
    assert all(t % 128 == 0 for t in out[:-1])
    return out


def _build_nc(C: int, reps: int = 1):
    import concourse.mybir as mybir
    import concourse.tile as tile
    from concourse import bacc
    from contextlib import nullcontext

    f32 = mybir.dt.float32
    bf16 = mybir.dt.bfloat16
    Silu = mybir.ActivationFunctionType.Silu

    assert C % 32 == 0
    NCOL = -(-C // 128)
    chunks = _chunks_of(C)

    nc = bacc.Bacc(None, target_bir_lowering=False)
    xT_d = nc.dram_tensor("xT", [K_TILES, 128, C], bf16, kind="ExternalInput")
    gw_d = nc.dram_tensor("gw", [H_TILES, 128, K_TILES, 128], bf16, kind="ExternalInput")
    uw_d = nc.dram_tensor("uw", [H_TILES, 128, K_TILES, 128], bf16, kind="ExternalInput")
    dw_d = nc.dram_tensor("dw", [H_TILES, 128, D_MODEL], bf16, kind="ExternalInput")
    cw_d = nc.dram_tensor("cwT", [128, NCOL], f32, kind="ExternalInput")
    y_d = nc.dram_tensor("y", [C, D_MODEL], bf16, kind="ExternalOutput")

    with tile.TileContext(nc) as tc:
        with (
            tc.tile_pool(name="xt", bufs=2) as xt_pool,
            tc.tile_pool(name="wgt", bufs=6) as wgt_pool,
            tc.tile_pool(name="dwsb", bufs=1) as dw_pool,
            tc.tile_pool(name="hb", bufs=1) as hb_pool,
            tc.tile_pool(name="sg", bufs=2) as sg_pool,
            tc.tile_pool(name="yo", bufs=4) as y_pool,
            tc.tile_pool(name="cw", bufs=1) as cw_pool,
            tc.tile_pool(name="ps1", bufs=2, space="PSUM") as ps1,
            tc.tile_pool(name="ps2", bufs=6, space="PSUM") as ps2,
        ):
            cw_sb = cw_pool.tile([128, NCOL], f32)
            nc.sync.dma_start(cw_sb[:], cw_d[:])
            # down-proj weights: fully SBUF-resident, loaded once (not per rep)
            dw_sb = dw_pool.tile([128, H_TILES, D_MODEL], bf16)
            for hi in range(H_TILES):
                nc.scalar.dma_start(dw_sb[:, hi, :], dw_d[hi])

            rep_ctx = tc.For_i(0, reps, 1) if reps > 1 else nullcontext()
            with rep_ctx:
              t0 = 0
              for TC in chunks:
                xt = xt_pool.tile([128, K_TILES, TC], bf16, tag="xt")
                # DRAM [k, d, t-slice] -> SBUF [d, k, t]
                nc.sync.dma_start(
                    xt[:], xT_d[:, :, t0:t0 + TC].transpose([1, 0, 2])
                )
                hbuf = hb_pool.tile([128, H_TILES, TC], bf16, tag="hbuf")

                groups = [512] * (TC // 512) + ([TC % 512] if TC % 512 else [])
                # ---- gate/up + SwiGLU, one 128-row tile of d_inner at a time
                for hi in range(H_TILES):
                    gw = wgt_pool.tile([128, K_TILES, 128], bf16, tag="w")
                    nc.sync.dma_start(gw[:], gw_d[hi])
                    uw = wgt_pool.tile([128, K_TILES, 128], bf16, tag="w")
                    nc.sync.dma_start(uw[:], uw_d[hi])
                    g0 = 0
                    for gsz in groups:
                        hs = slice(g0, g0 + gsz)
                        pg = ps1.tile([128, gsz], f32, tag="p1", name="pg", padded_shape=[128, 512])
                        for k in range(K_TILES):
                            nc.tensor.matmul(pg[:], gw[:, k, :], xt[:, k, hs],
                                             start=(k == 0), stop=(k == K_TILES - 1))
                        pu = ps1.tile([128, gsz], f32, tag="p1", name="pu", padded_shape=[128, 512])
                        for k in range(K_TILES):
                            nc.tensor.matmul(pu[:], uw[:, k, :], xt[:, k, hs],
                                             start=(k == 0), stop=(k == K_TILES - 1))
                        sg = sg_pool.tile([128, gsz], f32, tag="sg", name="sg", padded_shape=[128, 512])
                        nc.scalar.activation(sg[:], pg[:], Silu)
                        nc.vector.tensor_mul(hbuf[:, hi, hs], sg[:], pu[:])
                        g0 += gsz

                # ---- down-projection: psum [tokens, dout-half], accumulate
                # over all 32 h-tiles; dw comes from SBUF (no DMA)
                subs = [(s * 128, min(128, TC - s * 128)) for s in range(-(-TC // 128))]
                for half in range(2):
                    ds_ = slice(half * 512, (half + 1) * 512)
                    for si in range(0, len(subs), 6):
                        batch = subs[si:si + 6]
                        yps = [ps2.tile([ssz, 512], f32, tag="yp", name="yp", padded_shape=[128, 512])
                               for (_, ssz) in batch]
                        for hi in range(H_TILES):
                            for j, (so, ssz) in enumerate(batch):
                                nc.tensor.matmul(
                                    yps[j][:], hbuf[:, hi, so:so + ssz], dw_sb[:, hi, ds_],
                                    start=(hi == 0), stop=(hi == H_TILES - 1))
                        for j, (so, ssz) in enumerate(batch):
                            gcol = (t0 + so) // 128
                            yt = y_pool.tile([ssz, 512], bf16, tag="yt", name="yt", padded_shape=[128, 512])
                            nc.vector.tensor_scalar_mul(
                                yt[:], yps[j][:], cw_sb[0:ssz, gcol:gcol + 1])
                            nc.scalar.dma_start(
                                y_d[t0 + so: t0 + so + ssz, ds_], yt[:])
                t0 += TC
    nc.finalize()
    return nc


_NC_CACHE: dict = {}


def _get_nc(C: int):
    if C not in _NC_CACHE:
        _NC_CACHE[C] = _build_nc(C)
    return _NC_CACHE[C]


def _route(x2d: np.ndarray, router_w: np.ndarray, router_b: np.ndarray):
    """fp64 router: returns (idx_per_expert, cw_per_expert) lists."""
    logits = x2d.astype(np.float64) @ router_w.astype(np.float64).T + router_b.astype(np.float64)
    m = logits.max(axis=-1, keepdims=True)
    p = np.exp(logits - m)
    p /= p.sum(axis=-1, keepdims=True)
    # top-2 (jax.lax.top_k picks largest; softmax is monotonic in logits)
    i1 = np.argmax(p, axis=-1)
    p_masked = p.copy()
    p_masked[np.arange(p.shape[0]), i1] = -1.0
    i2 = np.argmax(p_masked, axis=-1)
    p1 = p[np.arange(p.shape[0]), i1]
    p2 = p[np.arange(p.shape[0]), i2]
    denom = p1 + p2
    w1 = p1 / denom
    w2 = p2 / denom
    idxs, cws = [], []
    for e in range(N_EXPERTS):
        sel1 = np.nonzero(i1 == e)[0]
        sel2 = np.nonzero(i2 == e)[0]
        idx = np.concatenate([sel1, sel2])
        cw = np.concatenate([w1[sel1], w2[sel2]])
        idxs.append(idx)
        cws.append(cw.astype(np.float32))
    return idxs, cws


def _prep_core_inputs(x2d, idxs, cws, gate_w, up_w, down_w, C):
    NCOL = -(-C // 128)
    in_maps = []
    for e in range(N_EXPERTS):
        idx = idxs[e]
        n = len(idx)
        xe = np.zeros((C, D_MODEL), np.float32)
        xe[:n] = x2d[idx]
        xT = np.ascontiguousarray(xe.T).astype(BF16).reshape(K_TILES, 128, C)
        gw = np.ascontiguousarray(
            gate_w[e].T.reshape(K_TILES, 128, H_TILES, 128).transpose(2, 1, 0, 3)).astype(BF16)
        uw = np.ascontiguousarray(
            up_w[e].T.reshape(K_TILES, 128, H_TILES, 128).transpose(2, 1, 0, 3)).astype(BF16)
        dw = np.ascontiguousarray(down_w[e].T).reshape(H_TILES, 128, D_MODEL).astype(BF16)
        cw = np.zeros((NCOL * 128,), np.float32)
        cw[:n] = cws[e]
        cwT = np.ascontiguousarray(cw.reshape(-1, 128).T)
        in_maps.append({"xT": xT, "gw": gw, "uw": uw, "dw": dw, "cwT": cwT})
    return in_maps


def kernel(x, router_w, router_b, gate_w, up_w, down_w):
    from concourse.bass_utils import run_bass_kernel_spmd

    x = np.asarray(x, dtype=np.float32)
    router_w = np.asarray(router_w, dtype=np.float32)
    router_b = np.asarray(router_b, dtype=np.float32)
    gate_w = np.asarray(gate_w, dtype=np.float32)
    up_w = np.asarray(up_w, dtype=np.float32)
    down_w = np.asarray(down_w, dtype=np.float32)

    B, S, D = x.shape
    x2d = x.reshape(B * S, D)
    idxs, cws = _route(x2d, router_w, router_b)
    max_n = max(len(i) for i in idxs)
    C = _capacity(max_n)

    nc = _get_nc(C)
    in_maps = _prep_core_inputs(x2d, idxs, cws, gate_w, up_w, down_w, C)
    res = run_bass_kernel_spmd(nc, in_maps, core_ids=list(range(N_EXPERTS)), trace=False)

    out = np.zeros((B * S, D_MODEL), np.float32)
    for e in range(N_EXPERTS):
        n = len(idxs[e])
        np.add.at(out, idxs[e], res.results[e]["y"][:n].astype(np.float32))
    return out.reshape(B, S, D_MODEL)


# revision 8
# speedup vs baseline: 1.1857x; 1.1857x over previous
"""MoE FFN (8 experts, top-2) on 8 TRN2 NeuronCores, expert-parallel.

Strategy:
  - Host: router (fp64 logits -> softmax -> top-2 -> renormalized combine
    weights), gather each expert's assigned tokens into a fixed capacity
    C=2048 (SPMD: one program, per-core inputs). The few tokens beyond
    capacity (~0.7% of slots for balanced routing) are computed on the
    host in fp32 and added into the output.
  - Core e: full SwiGLU FFN for expert e over its C tokens in bf16
    (fp32 PSUM accumulation), combine-weight scaling on device;
    outputs [C, 1024] bf16.
  - Host: scatter-add per-expert outputs back into [B, S, D].

Why bf16 + C=2048: PE matmuls cost ~(moving_rows + 134) cycles each
(the per-instruction Ldweights/decode tax is not overlapped on TRN2),
so the kernel keeps every matmul at the PSUM-bank maximum of 512 moving
rows and minimizes instruction count. bf16 runs at the same PE rate as
fp32r, halves DMA, and keeps rel err ~4e-3 (tolerance 2e-2). down_w
stays fully SBUF-resident across the rep loop; gate/up weights stream
per chunk; output DMAs ride the Activation HWDGE queue.

Layouts (host-prepared, DMA-friendly):
  xT   [8, 128, C]       x[idx].T split along d into 8 k-tiles
  gw/uw[32, 128, 8, 128] gate/up ^T tiled: [h_tile][d_sub][k][h]
  dw   [32, 128, 1024]   down^T tiled:     [h_tile][h_sub][dout]
  cwT  [128, C/128]      combine weights, partition-major
"""
import sys, os
for p in ("/opt/trn_rl_repo", os.path.join(os.path.dirname(os.path.abspath(__file__)))):
    if p not in sys.path:
        sys.path.insert(0, p)
import numpy as np
import ml_dtypes

BF16 = ml_dtypes.bfloat16
D_MODEL = 1024
D_INNER = 4096
N_EXPERTS = 8
TOP_K = 2
H_TILES = D_INNER // 128  # 32
K_TILES = D_MODEL // 128  # 8
MAXCH = 1056              # token chunk cap (hbuf SBUF limit)
C_DEVICE = 2048           # device capacity per core; overflow -> host fp32


def _capacity(max_n: int) -> int:
    return min(max(256, ((max_n + 31) // 32) * 32), C_DEVICE)


def _chunks_of(C: int):
    # every chunk except the last must be a multiple of 128 so chunk starts
    # stay 128-aligned (cw column mapping + phase2 sub tiling rely on it)
    n = -(-C // MAXCH)
    while True:
        base = (C // n) // 128 * 128
        last = C - base * (n - 1)
        if 0 < last <= MAXCH:
            break
        n += 1
    out = [base] * (n - 1) + [last]
    assert sum(out) == C and all(0 < t <= MAXCH for t in out)
    assert all(t % 128 == 0 for t in out[:-1])
    return out


def _build_nc(C: int, reps: int = 1):
    import concourse.mybir as mybir
    import concourse.tile as tile
    from concourse import bacc
    from contextlib import nullcontext

    f32 = mybir.dt.float32
    bf16 = mybir.dt.bfloat16
    Silu = mybir.ActivationFunctionType.Silu

    assert C % 32 == 0
    NCOL = -(-C // 128)
    chunks = _chunks_of(C)

    nc = bacc.Bacc(None, target_bir_lowering=False)
    xT_d = nc.dram_tensor("xT", [K_TILES, 128, C], bf16, kind="ExternalInput")
    gw_d = nc.dram_tensor("gw", [H_TILES, 128, K_TILES, 128], bf16, kind="ExternalInput")
    uw_d = nc.dram_tensor("uw", [H_TILES, 128, K_TILES, 128], bf16, kind="ExternalInput")
    dw_d = nc.dram_tensor("dw", [H_TILES, 128, D_MODEL], bf16, kind="ExternalInput")
    cw_d = nc.dram_tensor("cwT", [128, NCOL], f32, kind="ExternalInput")
    y_d = nc.dram_tensor("y", [C, D_MODEL], bf16, kind="ExternalOutput")

    with tile.TileContext(nc) as tc:
        with (
            tc.tile_pool(name="xt", bufs=2) as xt_pool,
            tc.tile_pool(name="wgt", bufs=8) as wgt_pool,
            tc.tile_pool(name="dwsb", bufs=1) as dw_pool,
            tc.tile_pool(name="hb", bufs=1) as hb_pool,
            tc.tile_pool(name="sg", bufs=2) as sg_pool,
            tc.tile_pool(name="yo", bufs=4) as y_pool,
            tc.tile_pool(name="cw", bufs=1) as cw_pool,
            tc.tile_pool(name="ps1", bufs=4, space="PSUM") as ps1,
            tc.tile_pool(name="ps2", bufs=4, space="PSUM") as ps2,
        ):
            cw_sb = cw_pool.tile([128, NCOL], f32)
            nc.sync.dma_start(cw_sb[:], cw_d[:])
            # down-proj weights: fully SBUF-resident, loaded once (not per rep)
            dw_sb = dw_pool.tile([128, H_TILES, D_MODEL], bf16)
            for hi in range(H_TILES):
                nc.sync.dma_start(dw_sb[:, hi, :], dw_d[hi])

            rep_ctx = tc.For_i(0, reps, 1) if reps > 1 else nullcontext()
            with rep_ctx:
              t0 = 0
              for TC in chunks:
                xt = xt_pool.tile([128, K_TILES, TC], bf16, tag="xt")
                # DRAM [k, d, t-slice] -> SBUF [d, k, t]
                nc.sync.dma_start(
                    xt[:], xT_d[:, :, t0:t0 + TC].transpose([1, 0, 2])
                )
                hbuf = hb_pool.tile([128, H_TILES, TC], bf16, tag="hbuf")

                groups = [512] * (TC // 512) + ([TC % 512] if TC % 512 else [])
                # ---- gate/up + SwiGLU, one 128-row tile of d_inner at a time
                for hi in range(H_TILES):
                    gw = wgt_pool.tile([128, K_TILES, 128], bf16, tag="w")
                    nc.sync.dma_start(gw[:], gw_d[hi])
                    uw = wgt_pool.tile([128, K_TILES, 128], bf16, tag="w")
                    nc.sync.dma_start(uw[:], uw_d[hi])
                    g0 = 0
                    for gsz in groups:
                        hs = slice(g0, g0 + gsz)
                        pg = ps1.tile([128, gsz], f32, tag="p1", name="pg", padded_shape=[128, 512])
                        for k in range(K_TILES):
                            nc.tensor.matmul(pg[:], gw[:, k, :], xt[:, k, hs],
                                             start=(k == 0), stop=(k == K_TILES - 1))
                        pu = ps1.tile([128, gsz], f32, tag="p1", name="pu", padded_shape=[128, 512])
                        for k in range(K_TILES):
                            nc.tensor.matmul(pu[:], uw[:, k, :], xt[:, k, hs],
                                             start=(k == 0), stop=(k == K_TILES - 1))
                        sg = sg_pool.tile([128, gsz], f32, tag="sg", name="sg", padded_shape=[128, 512])
                        nc.scalar.activation(sg[:], pg[:], Silu)
                        nc.vector.tensor_mul(hbuf[:, hi, hs], sg[:], pu[:])
                        g0 += gsz

                # ---- down-projection: psum [tokens, dout-half], accumulate
                # over all 32 h-tiles; dw comes from SBUF (no DMA)
                subs = [(s * 128, min(128, TC - s * 128)) for s in range(-(-TC // 128))]
                for half in range(2):
                    ds_ = slice(half * 512, (half + 1) * 512)
                    for si in range(0, len(subs), 4):
                        batch = subs[si:si + 4]
                        yps = [ps2.tile([ssz, 512], f32, tag="yp", name="yp", padded_shape=[128, 512])
                               for (_, ssz) in batch]
                        for hi in range(H_TILES):
                            for j, (so, ssz) in enumerate(batch):
                                nc.tensor.matmul(
                                    yps[j][:], hbuf[:, hi, so:so + ssz], dw_sb[:, hi, ds_],
                                    start=(hi == 0), stop=(hi == H_TILES - 1))
                        for j, (so, ssz) in enumerate(batch):
                            gcol = (t0 + so) // 128
                            yt = y_pool.tile([ssz, 512], bf16, tag="yt", name="yt", padded_shape=[128, 512])
                            nc.vector.tensor_scalar_mul(
                                yt[:], yps[j][:], cw_sb[0:ssz, gcol:gcol + 1])
                            nc.scalar.dma_start(
                                y_d[t0 + so: t0 + so + ssz, ds_], yt[:])
                t0 += TC
    nc.finalize()
    return nc


_NC_CACHE: dict = {}


def _get_nc(C: int):
    if C not in _NC_CACHE:
        _NC_CACHE[C] = _build_nc(C)
    return _NC_CACHE[C]


def _route(x2d: np.ndarray, router_w: np.ndarray, router_b: np.ndarray):
    """fp64 router: returns (idx_per_expert, cw_per_expert) lists."""
    logits = x2d.astype(np.float64) @ router_w.astype(np.float64).T + router_b.astype(np.float64)
    m = logits.max(axis=-1, keepdims=True)
    p = np.exp(logits - m)
    p /= p.sum(axis=-1, keepdims=True)
    # top-2 (jax.lax.top_k picks largest; softmax is monotonic in logits)
    i1 = np.argmax(p, axis=-1)
    p_masked = p.copy()
    p_masked[np.arange(p.shape[0]), i1] = -1.0
    i2 = np.argmax(p_masked, axis=-1)
    p1 = p[np.arange(p.shape[0]), i1]
    p2 = p[np.arange(p.shape[0]), i2]
    denom = p1 + p2
    w1 = p1 / denom
    w2 = p2 / denom
    idxs, cws = [], []
    for e in range(N_EXPERTS):
        sel1 = np.nonzero(i1 == e)[0]
        sel2 = np.nonzero(i2 == e)[0]
        idx = np.concatenate([sel1, sel2])
        cw = np.concatenate([w1[sel1], w2[sel2]])
        idxs.append(idx)
        cws.append(cw.astype(np.float32))
    return idxs, cws


def _prep_core_inputs(x2d, idxs, cws, gate_w, up_w, down_w, C):
    NCOL = -(-C // 128)
    in_maps = []
    for e in range(N_EXPERTS):
        idx = idxs[e]
        n = min(len(idx), C)
        xe = np.zeros((C, D_MODEL), np.float32)
        xe[:n] = x2d[idx[:n]]
        xT = np.ascontiguousarray(xe.T).astype(BF16).reshape(K_TILES, 128, C)
        gw = np.ascontiguousarray(
            gate_w[e].T.reshape(K_TILES, 128, H_TILES, 128).transpose(2, 1, 0, 3)).astype(BF16)
        uw = np.ascontiguousarray(
            up_w[e].T.reshape(K_TILES, 128, H_TILES, 128).transpose(2, 1, 0, 3)).astype(BF16)
        dw = np.ascontiguousarray(down_w[e].T).reshape(H_TILES, 128, D_MODEL).astype(BF16)
        cw = np.zeros((NCOL * 128,), np.float32)
        cw[:n] = cws[e][:n]
        cwT = np.ascontiguousarray(cw.reshape(-1, 128).T)
        in_maps.append({"xT": xT, "gw": gw, "uw": uw, "dw": dw, "cwT": cwT})
    return in_maps


def _ffn_host(x, gate_w_e, up_w_e, down_w_e):
    g = x @ gate_w_e.T
    u = x @ up_w_e.T
    h = (g / (1.0 + np.exp(-g))) * u
    return h @ down_w_e.T


def kernel(x, router_w, router_b, gate_w, up_w, down_w):
    from concourse.bass_utils import run_bass_kernel_spmd

    x = np.asarray(x, dtype=np.float32)
    router_w = np.asarray(router_w, dtype=np.float32)
    router_b = np.asarray(router_b, dtype=np.float32)
    gate_w = np.asarray(gate_w, dtype=np.float32)
    up_w = np.asarray(up_w, dtype=np.float32)
    down_w = np.asarray(down_w, dtype=np.float32)

    B, S, D = x.shape
    x2d = x.reshape(B * S, D)
    idxs, cws = _route(x2d, router_w, router_b)
    max_n = max(len(i) for i in idxs)
    C = _capacity(max_n)

    nc = _get_nc(C)
    in_maps = _prep_core_inputs(x2d, idxs, cws, gate_w, up_w, down_w, C)
    res = run_bass_kernel_spmd(nc, in_maps, core_ids=list(range(N_EXPERTS)), trace=False)

    out = np.zeros((B * S, D_MODEL), np.float32)
    for e in range(N_EXPERTS):
        n = min(len(idxs[e]), C)
        np.add.at(out, idxs[e][:n], res.results[e]["y"][:n].astype(np.float32))
        if len(idxs[e]) > C:
            # capacity overflow: host fp32 fallback for the tail tokens
            oi = idxs[e][C:]
            ocw = cws[e][C:]
            yo = _ffn_host(x2d[oi], gate_w[e], up_w[e], down_w[e])
            np.add.at(out, oi, ocw[:, None] * yo)
    return out.reshape(B, S, D_MODEL)
